# revision 30
# baseline (speedup 1.0000x reference)
"""TRN2 Bass/Tile kernel: 16-head causal multi-head attention.

Problem: x[2,2048,1024], 16 heads x 64, causal softmax attention + out-proj.

Sharding (8 cores): core = b*4 + g  (b = batch 0..1, g = head-group 0..3).
Each core computes heads [4g, 4g+4) for batch b and the partial
out-projection  ctx_g @ Wo[g*256:(g+1)*256, :]  -> [2048, 1024] (bf16).
Host upcasts, sums the 4 partials per batch and adds bo.

On-device layout is fully "transposed" (feature-major):
  xT   [128, 8, 2048]  : xT[p, kt, s]  = x[b, s, kt*128+p]
  QT/KT[128, 2, 2048]  : QT[p, t, s]   = Q^T[t*128+p, s]   (d' = h*64+j on partitions)
  S^T  [128k, 512q]    : per (head, q-chunk, k-tile) block = K @ Q^T
  softmax: no max-subtraction (scores are O(1) by construction: exp is safe);
  denominators via a ones-column appended to V (row 64 of the ctx psum);
  ctx^T [128, 2, 2048] feeds the out-projection directly as lhsT.

Schedule (v5):
  - Head pairs (h0,h1)/(h2,h3) write one [128, 2, 512] two-bank S^T psum
    tile; their matmuls row-tile the PE array concurrently (K=64 halves)
    and ONE exp per pair halves ScalarE's per-op fixed cost.
  - ctx accumulates into ONE persistent 4-bank psum tile [65, 4, 512]
    so the denominator row (p=64) of all 4 heads evicts in ONE DVE op.
  - Normalization: den -> SBUF (1 op), f32 ones-matmul broadcasts den to
    64 partitions, reciprocal_approx_fast runs on 64 lanes (not 1), then
    one DVE mul per head.  No Log/Exp table switches, no 1-lane recips.
  - Causal mask (affine_select) trimmed to the 128-wide crossing block.
  - Output stored bf16 (halves the 8MB out DMA); host sums in f32.
  - Warmup: ~64 N=128 dummy matmuls from memset tiles (no DMA dep) keep
    the PE HAM at 2.4GHz through the initial DMA; tails emitted one
    chunk late, interleaved between projection chains.
"""

import os
import sys

for _p in ("/opt/trn_rl_repo",):
    if _p not in sys.path:
        sys.path.insert(0, _p)

import numpy as np

import concourse.bass as bass
import concourse.mybir as mybir
import concourse.tile as tile
from concourse import bacc
from concourse.bass import ts
from concourse.bass_utils import run_bass_kernel_spmd

B, S, D, H, HD = 2, 2048, 1024, 16, 64
GROUPS, HPG, DG = 4, 4, 256  # head groups, heads/group, group width
KT = D // 128  # 8 k-tiles over D
ST = S // 128  # 16 s-tiles
CH = 512  # q-chunk width
QCH = S // CH  # 4 q-chunks
PIPE = 2  # ctx matmuls trail the S^T/exp stream by this many k-steps
F32 = mybir.dt.float32

_MM_DT_NAME = os.environ.get("BASS_MM_DT", "bf16")
MM_DT = {
    "f32r": mybir.dt.float32r,
    "f32": mybir.dt.float32,
    "bf16": mybir.dt.bfloat16,
}[_MM_DT_NAME]
WARMUP = int(os.environ.get("BASS_WARMUP", "48"))
TAILFILL = int(os.environ.get("BASS_TAILFILL", "26"))
WU_N = int(os.environ.get("BASS_WU_N", "128"))


def _np_dt():
    import ml_dtypes

    return ml_dtypes.bfloat16 if _MM_DT_NAME == "bf16" else np.float32


def build_kernel_body(nc, tc, io):
    Exp = mybir.ActivationFunctionType.Exp

    consts = tc.alloc_tile_pool(name="consts", bufs=1)
    acts = tc.alloc_tile_pool(name="acts", bufs=1)
    work = tc.alloc_tile_pool(name="work", bufs=2)
    small = tc.alloc_tile_pool(name="small", bufs=2)
    psum = tc.alloc_tile_pool(name="psum", bufs=1, space="PSUM")

    # ---- on-chip constants (no DMA dependency: warmup starts at t=0) ----
    wu_sb = consts.tile([128, 128], MM_DT)  # K=128 warmup operand + bcast ones
    nc.vector.memset(wu_sb, 1.0)

    # ---- constant loads (hot-first emission order) -----------------------
    wq_sb = consts.tile([128, KT, DG], MM_DT)
    nc.sync.dma_start(out=wq_sb, in_=io["wq"])
    wk_sb = consts.tile([128, KT, DG], MM_DT)
    nc.sync.dma_start(out=wk_sb, in_=io["wk"])
    xt_sb = consts.tile([128, KT, S], MM_DT)
    for kt in range(KT):  # chunk-0 columns first, then the rest per k-tile
        nc.sync.dma_start(out=xt_sb[:, kt, 0:CH], in_=io["xt"][:, kt, 0:CH])
    bq_sb = consts.tile([128, 2], F32)
    nc.sync.dma_start(out=bq_sb, in_=io["bq"])
    bk_sb = consts.tile([128, 2], F32)
    nc.sync.dma_start(out=bk_sb, in_=io["bk"])
    wv_sb = consts.tile([128, KT, DG], MM_DT)
    nc.sync.dma_start(out=wv_sb, in_=io["wv"])
    vb_sb = consts.tile([128, HPG, HD], F32)
    nc.sync.dma_start(out=vb_sb, in_=io["vb"])
    for kt in range(KT):
        nc.sync.dma_start(out=xt_sb[:, kt, CH:S], in_=io["xt"][:, kt, CH:S])
    wo_sb = consts.tile([128, 2, 1024], MM_DT)
    nc.sync.dma_start(out=wo_sb, in_=io["wo"])

    # ---- persistent activations ----------------------------------------
    qt_sb = acts.tile([128, 2, S], MM_DT)  # Q^T (pre-scaled by 1/8 via host W/b)
    kt_sb = acts.tile([128, 2, S], MM_DT)  # K^T
    v_sb = acts.tile([128, ST, HPG, HD], MM_DT)  # V blocks
    ctxT_sb = acts.tile([128, 2, S], MM_DT)  # normalized ctx^T

    # ctx accumulator: pair pr in bank pr, head 2pr+sub at partitions 64*sub
    # (matches ctxT layout exactly); denominators col-tiled at partition 32h
    ctx2_ps = psum.tile([128, 2, CH], F32, tag="ctx", bufs=1, name="ctx2_ps")
    den_ps = psum.tile([128, CH], F32, tag="denp", bufs=1, name="den_ps")

    # ---- PE warmup: full-K matmuls flip the HAM clock gate to 8/8 -------
    wu_ps = psum.tile([128, WU_N], F32, tag="sT", bufs=2, name="wu_ps")
    for r in range(WARMUP):
        nc.tensor.matmul(
            wu_ps, lhsT=wu_sb, rhs=wu_sb[:, 0:WU_N], start=True, stop=True
        )

    def proj_chains(c):
        """Q^T/K^T chunk c + V s-tiles of chunk c as a list of emit-thunks."""
        chains = []
        for t in range(2):
            for which, w_sb, b_sb, dst in (
                ("q", wq_sb, bq_sb, qt_sb),
                ("k", wk_sb, bk_sb, kt_sb),
            ):

                def chain(t=t, w_sb=w_sb, b_sb=b_sb, dst=dst, which=which):
                    ps = psum.tile([128, CH], F32, tag="sT", bufs=2, name=f"{which}_ps{c}{t}")
                    for kt in range(KT):
                        nc.tensor.matmul(
                            ps[:, 0:CH],
                            lhsT=w_sb[:, kt, ts(t, 128)],
                            rhs=xt_sb[:, kt, ts(c, CH)],
                            start=(kt == 0),
                            stop=(kt == KT - 1),
                        )
                    nc.vector.tensor_scalar_add(
                        out=dst[:, t, ts(c, CH)], in0=ps[:, 0:CH], scalar1=b_sb[:, t : t + 1]
                    )

                chains.append(chain)
        for st in range(4 * c, 4 * c + 4):

            def chain(st=st):
                ps = psum.tile([128, DG], F32, tag="sT", bufs=2, name=f"v_ps{st}")
                for kt in range(KT):
                    nc.tensor.matmul(
                        ps,
                        lhsT=xt_sb[:, kt, ts(st, 128)],
                        rhs=wv_sb[:, kt, :],
                        start=(kt == 0),
                        stop=(kt == KT - 1),
                    )
                nc.vector.tensor_add(
                    out=v_sb[:, st, :, :],
                    in0=ps.rearrange("p (h j) -> p h j", h=HPG),
                    in1=vb_sb,
                )

            chains.append(chain)
        return chains

    def proj_pieces(c):
        """proj chains for chunk c as ~1us emit-thunks on the aux psum bank
        (never touches the sT tag, so score double-buffering is untouched)."""
        pieces = []
        for t in range(2):
            for which, w_sb, b_sb, dst in (
                ("q", wq_sb, bq_sb, qt_sb),
                ("k", wk_sb, bk_sb, kt_sb),
            ):
                shared = {}

                def p1(t=t, w_sb=w_sb, shared=shared, which=which):
                    ps = psum.tile(
                        [128, CH], F32, tag="aux", bufs=1, name=f"{which}_ps"
                    )
                    shared["ps"] = ps
                    for kt in range(4):
                        nc.tensor.matmul(
                            ps,
                            lhsT=w_sb[:, kt, ts(t, 128)],
                            rhs=xt_sb[:, kt, ts(c, CH)],
                            start=(kt == 0),
                            stop=False,
                        )

                def p2(t=t, w_sb=w_sb, b_sb=b_sb, dst=dst, shared=shared):
                    ps = shared["ps"]
                    for kt in range(4, KT):
                        nc.tensor.matmul(
                            ps,
                            lhsT=w_sb[:, kt, ts(t, 128)],
                            rhs=xt_sb[:, kt, ts(c, CH)],
                            start=False,
                            stop=(kt == KT - 1),
                        )
                    nc.vector.tensor_scalar_add(
                        out=dst[:, t, ts(c, CH)], in0=ps, scalar1=b_sb[:, t : t + 1]
                    )

                pieces += [p1, p2]
        for st in range(4 * c, 4 * c + 4):

            def pv(st=st):
                ps = psum.tile([128, DG], F32, tag="aux", bufs=1, name=f"v_ps{st}")
                for kt in range(KT):
                    nc.tensor.matmul(
                        ps,
                        lhsT=xt_sb[:, kt, ts(st, 128)],
                        rhs=wv_sb[:, kt, :],
                        start=(kt == 0),
                        stop=(kt == KT - 1),
                    )
                nc.vector.tensor_add(
                    out=v_sb[:, st, :, :],
                    in0=ps.rearrange("p (h j) -> p h j", h=HPG),
                    in1=vb_sb,
                )

            pieces.append(pv)
        return pieces

    def oproj_pieces(c):
        """out-projection for chunk c as 2 thunks per s-tile (aux psum)."""
        pieces = []
        for st in range(4 * c, 4 * c + 4):
            shared = {}
            for nch in range(2):

                def p(st=st, shared=shared, c=c, nch=nch):
                    if nch == 0:
                        shared["o_sb"] = work.tile(
                            [128, 1024], MM_DT, tag="osb", bufs=3, name="o_sb"
                        )
                    o_sb = shared["o_sb"]
                    ps = psum.tile([128, CH], F32, tag="aux", bufs=1, name="o_ps")
                    for t in range(2):
                        nc.tensor.matmul(
                            ps,
                            lhsT=ctxT_sb[:, t, ts(st, 128)],
                            rhs=wo_sb[:, t, ts(nch, CH)],
                            start=(t == 0),
                            stop=(t == 1),
                        )
                    if c == QCH - 1 and nch == 0:
                        # tail chunk: ScalarE is idle after the last exp
                        nc.scalar.copy(out=o_sb[:, ts(nch, CH)], in_=ps)
                    else:
                        nc.vector.tensor_copy(out=o_sb[:, ts(nch, CH)], in_=ps)
                    if nch == 1:
                        nc.sync.dma_start(out=io["out"][ts(st, 128), :], in_=o_sb)

                pieces.append(p)
        return pieces

    def emit_attn(c, fillers=(), prio=()):
        """S^T/exp per head-pair, ctx matmuls trailing by PIPE k-steps;
        filler thunks are drip-fed between tiles to soak up the PE slack
        under the exp-paced ScalarE stream.  prio thunks (the previous
        chunk's norm chain) MUST all be emitted before ctx(0) reuses the
        ctx psum, i.e. within the first PIPE steps."""
        from collections import deque

        fl = deque(fillers)
        pq = deque(prio)
        nkt = (c + 1) * (CH // 128)
        exps = [[None] * nkt for _ in range(2)]  # per pair

        def scores(i):
            off = max(0, 128 * i - CH * c)  # first unmasked column of this k-tile
            for pr in range(2):  # head pair (2*pr, 2*pr+1) -> tile t=pr
                sT_ps = psum.tile([128, 2, CH], F32, tag="sT", bufs=2, name="sT_ps")
                for sub in range(2):
                    pb = sub * 64
                    nc.tensor.matmul(
                        sT_ps[:, sub, off:CH],
                        lhsT=kt_sb[pb : pb + HD, pr, ts(i, 128)],
                        rhs=qt_sb[pb : pb + HD, pr, c * CH + off : (c + 1) * CH],
                        start=True,
                        stop=True,
                    )
                e = work.tile([128, 2, CH], MM_DT, tag="exp", bufs=8, name="e")
                nc.scalar.activation(
                    out=e[:, :, off:CH], in_=sT_ps[:, :, off:CH], func=Exp
                )
                if 128 * i + 128 > CH * c + off:  # crosses the diagonal: mask
                    # only the 128-wide crossing block needs masking;
                    # columns beyond it are fully unmasked
                    mw = min(128, CH - off)
                    nc.gpsimd.affine_select(
                        out=e[:, :, off : off + mw],
                        in_=e[:, :, off : off + mw],
                        pattern=[[0, 2], [1, mw]],
                        base=0,
                        channel_multiplier=-1,
                        compare_op=mybir.AluOpType.is_ge,
                        fill=0.0,
                    )
                exps[pr][i] = (e, off)

        def ctx(i):
            # 2 heads col-tiled per bank (concurrent on array col halves);
            # start=True clears has_written per written region, so each
            # head's first matmul of the chunk needs it
            for pr in range(2):
                e, off = exps[pr][i]
                for sub in range(2):
                    nc.tensor.matmul(
                        ctx2_ps[64 * sub : 64 * sub + 64, pr, off:CH],
                        lhsT=v_sb[:, i, 2 * pr + sub, :],
                        rhs=e[:, sub, off:CH],
                        start=(i == 0),
                        stop=(i == nkt - 1),
                        tile_position=(0, 64 * sub),
                    )
            # denominators: 4-way col-tiled M=1 matmuls, head h at partition 32h
            for h in range(HPG):
                e, off = exps[h // 2][i]
                nc.tensor.matmul(
                    den_ps[32 * h : 32 * h + 1, off:CH],
                    lhsT=wu_sb[:, 0:1],
                    rhs=e[:, h % 2, off:CH],
                    start=(i == 0),
                    stop=(i == nkt - 1),
                    tile_position=(0, 32 * h),
                )

        steps = nkt + PIPE
        for i in range(steps):
            if i < nkt:
                scores(i)
            if i == 0:
                for _ in range(2):
                    if pq:
                        pq.popleft()()
            elif pq:  # step 1: flush remaining prio (before ctx(0) at step 2)
                while pq:
                    pq.popleft()()
            if i >= PIPE:
                ctx(i - PIPE)
            if fl and not pq:
                k = min(3, max(1, -(-len(fl) // (steps - i))))
                for _ in range(k):
                    if fl:
                        fl.popleft()()
        while fl:
            fl.popleft()()

    def tail_norm_parts(c):
        """Denominator eviction + normalize thunks for chunk c."""
        parts = []
        den_sb = small.tile([97, CH], MM_DT, tag="den", name="den_sb")

        def den_evict():
            # denominators live at partitions 0/32/64/96 of den_ps; DVE can't
            # shift partitions, so evict the whole 97-row band in one op
            nc.vector.tensor_copy(out=den_sb, in_=den_ps[0:97, :])

        parts.append(den_evict)
        for pr in range(2):

            def norm(pr=pr, c=c):
                bc_ps = psum.tile([128, CH], F32, tag="aux", bufs=1, name="bc_ps")
                for sub in range(2):
                    p = 32 * (2 * pr + sub)  # weight and fmap must share start partition
                    nc.tensor.matmul(
                        bc_ps[64 * sub : 64 * sub + 64, :],
                        lhsT=wu_sb[p : p + 1, 0:HD],
                        rhs=den_sb[p : p + 1, :],
                        start=True,
                        stop=True,
                        tile_position=(p, 64 * sub),
                    )
                rcp_sb = small.tile([128, CH], F32, tag="rcp", name="rcp_sb")
                nc.vector.reciprocal_approx_fast(out=rcp_sb, in_=bc_ps)
                nc.vector.tensor_mul(
                    out=ctxT_sb[:, pr, ts(c, CH)],
                    in0=ctx2_ps[:, pr, :],
                    in1=rcp_sb,
                )

            parts.append(norm)
        return parts

    def run_all(thunks):
        for th in thunks:
            th()

    # ---- pipeline: attention phases run back-to-back; the previous
    # chunk's norm chain enters as priority fillers, projections and
    # out-projections drip in as regular fillers under the exp pace ------
    run_all(proj_chains(0))
    emit_attn(0, proj_pieces(1))
    emit_attn(1, oproj_pieces(0) + proj_pieces(2), prio=tail_norm_parts(0))
    emit_attn(2, oproj_pieces(1) + proj_pieces(3), prio=tail_norm_parts(1))
    emit_attn(3, oproj_pieces(2), prio=tail_norm_parts(2))
    # filler matmuls bridge the final norm's DVE latency so the PE stays
    # warm for the tail out-projections
    tf_ps = psum.tile([128, WU_N], F32, tag="sT", bufs=2, name="tf_ps")
    for r in range(TAILFILL):
        nc.tensor.matmul(
            tf_ps, lhsT=wu_sb, rhs=wu_sb[:, 0:WU_N], start=True, stop=True
        )
    run_all(tail_norm_parts(3))
    run_all(oproj_pieces(3))

    psum.release()
    small.release()
    work.release()
    acts.release()
    consts.release()


def build_nc():
    nc = bacc.Bacc("TRN2", target_bir_lowering=False, debug=False)
    io = {
        "xt": nc.dram_tensor("xt", [128, KT, S], MM_DT, kind="ExternalInput").ap(),
        "wq": nc.dram_tensor("wq", [128, KT, DG], MM_DT, kind="ExternalInput").ap(),
        "wk": nc.dram_tensor("wk", [128, KT, DG], MM_DT, kind="ExternalInput").ap(),
        "wv": nc.dram_tensor("wv", [128, KT, DG], MM_DT, kind="ExternalInput").ap(),
        "wo": nc.dram_tensor("wo", [128, 2, 1024], MM_DT, kind="ExternalInput").ap(),
        "bq": nc.dram_tensor("bq", [128, 2], F32, kind="ExternalInput").ap(),
        "bk": nc.dram_tensor("bk", [128, 2], F32, kind="ExternalInput").ap(),
        "vb": nc.dram_tensor("vb", [128, HPG, HD], F32, kind="ExternalInput").ap(),
        "out": nc.dram_tensor("out", [S, D], MM_DT, kind="ExternalOutput").ap(),
    }
    with tile.TileContext(nc) as tc, nc.allow_low_precision(
        reason="reduced-precision matmul operand pipeline; accumulation stays fp32"
    ):
        build_kernel_body(nc, tc, io)
    nc.compile()
    return nc


_NC = None


def get_nc():
    global _NC
    if _NC is None:
        _NC = build_nc()
    return _NC


def _tile_rows(a, p=128):
    """[R, N] -> [128, R//128, N] with row r = kt*128 + p."""
    r, n = a.shape
    return np.ascontiguousarray(a.reshape(r // p, p, n).transpose(1, 0, 2)).astype(
        _np_dt()
    )


def shard_inputs(x, Wq, bq, Wk, bk, Wv, bv, Wo, bo):
    scale = 1.0 / np.sqrt(np.float32(HD))
    in_maps = []
    for core in range(8):
        b, g = divmod(core, GROUPS)
        sl = slice(g * DG, (g + 1) * DG)
        vb = np.ascontiguousarray(
            np.broadcast_to(bv[sl].reshape(HPG, HD)[None], (128, HPG, HD))
        ).astype(np.float32)
        in_maps.append(
            {
                "xt": _tile_rows(np.ascontiguousarray(x[b].T)),
                "wq": _tile_rows(np.ascontiguousarray(Wq[:, sl]) * scale),
                "wk": _tile_rows(np.ascontiguousarray(Wk[:, sl])),
                "wv": _tile_rows(np.ascontiguousarray(Wv[:, sl])),
                "wo": _tile_rows(np.ascontiguousarray(Wo[sl, :])),
                "bq": np.ascontiguousarray((bq[sl] * scale).reshape(2, 128).T),
                "bk": np.ascontiguousarray(bk[sl].reshape(2, 128).T),
                "vb": vb,
            }
        )
    return in_maps


LAST_RESULT = None


def kernel(**inputs):
    global LAST_RESULT
    inputs = {k: np.asarray(v) for k, v in inputs.items()}
    nc = get_nc()
    in_maps = shard_inputs(**inputs)
    trace = bool(int(os.environ.get("BASS_KERNEL_TRACE", "0")))
    res = run_bass_kernel_spmd(nc, in_maps, core_ids=list(range(8)), trace=trace)
    LAST_RESULT = res
    parts = [res.results[c]["out"].astype(np.float32) for c in range(8)]
    out = np.stack(
        [
            parts[0] + parts[1] + parts[2] + parts[3],
            parts[4] + parts[5] + parts[6] + parts[7],
        ]
    )
    return (out + inputs["bo"]).astype(np.float32)
